# revision 7
# baseline (speedup 1.0000x reference)
"""Distributed Bass attention-head kernel for one TRN2 chip (8 NeuronCores).

Problem: x[8192,1024], Wk/Wq/Wv[64,1024] ->
  out  = softmax((x Wq^T)(x Wk^T)^T / sqrt(64)) @ (x Wv^T)   [8192, 64]
  obj  = pre-softmax affinity row 1                            [1, 8192]

Sharding: sequence-parallel. Each core owns 1024 rows of x, computes its
local q^T/k^T/v, all-gathers K^T and V' (V with a ones column appended so
the PV matmul also produces the softmax denominator), then runs a
flash-style loop over 64 kv chunks:
  S^T[kv=128, q=1024] = K^T_chunk^T @ q^T   (scale folded into Wq)
  P = exp(S^T)  (no max subtraction needed: |scores| <~ 8 in fp32)
  acc[65, q] += V'_chunk^T @ P              (row 64 = denominator)
Epilogue transposes acc via the PE and divides by the denominator.
"""

import os

import numpy as np
import ml_dtypes

import concourse.bass as bass
import concourse.tile as tile
from concourse import bacc, mybir
from concourse.bass import ts, ds
from concourse.bass_utils import run_bass_kernel_spmd
from concourse.masks import make_identity

T, C, H = 8192, 1024, 64
NCORES = 8
TLOC = T // NCORES            # 1024 rows of x per core
SCALE = H ** -0.5
NKV = T // 128                # 64 kv chunks of 128
NCC = C // 128                # 8 contraction chunks of 128
KLEN = H * TLOC               # elems of k^T [64, 1024] in the gather buffer
VLEN = TLOC * (H + 1)         # elems of v' [1024, 65] in the gather buffer
CC_LEN = KLEN + VLEN

BF = mybir.dt.bfloat16
F32 = mybir.dt.float32

LAST_EXEC_TIME_NS = None
_CACHED = {}


def _install_profile_hook():
    """Make trace=True work on the bare axon agent image.

    concourse's axon trace path reads the NTFF hook via
    ``antenv.axon_hooks``; on this image that module is absent, so
    synthesize it and register the ctypes-based hook from trn_boot.
    """
    import sys
    import types

    try:
        from antenv.axon_hooks import get_axon_ntff_profile_hook  # noqa: F401

        return True
    except ImportError:
        pass
    try:
        import antenv
        from trn_agent_boot.trn_boot import _ntff_profile_via_ctypes

        so_path = "/opt/axon/libaxon_pjrt.so"
        if not os.path.exists(so_path):
            return False
        hook = _ntff_profile_via_ctypes(so_path)
        if hook is None:
            return False
        mod = types.ModuleType("antenv.axon_hooks")
        mod._hook = hook
        mod.get_axon_ntff_profile_hook = lambda: mod._hook
        mod.set_axon_ntff_profile_hook = lambda h: setattr(mod, "_hook", h)
        sys.modules["antenv.axon_hooks"] = mod
        antenv.axon_hooks = mod
        return True
    except Exception:
        return False


def build_nc():
    nc = bacc.Bacc(None, debug=False, num_devices=NCORES)

    xT = nc.declare_dram_parameter("xT", [C, TLOC], BF, isOutput=False)
    wq = nc.declare_dram_parameter("wqT", [C, H], BF, isOutput=False)
    wk = nc.declare_dram_parameter("wkT", [C, H], BF, isOutput=False)
    wv = nc.declare_dram_parameter("wvT", [C, H], BF, isOutput=False)
    out_e = nc.declare_dram_parameter("out", [TLOC, H], F32, isOutput=True)
    obj_e = nc.declare_dram_parameter("obj", [128, NKV], F32, isOutput=True)

    cc_in = nc.dram_tensor("cc_in", [CC_LEN], BF)
    cc_out = nc.dram_tensor("cc_out", [NCORES * CC_LEN], BF, addr_space="Shared")

    with tile.TileContext(nc) as tc:
        with (
            tc.tile_pool(name="const", bufs=1) as constp,
            tc.tile_pool(name="xts", bufs=NCC) as xpool,
            tc.tile_pool(name="wts", bufs=1) as wpool,
            tc.tile_pool(name="proj", bufs=1) as projpool,
            tc.tile_pool(name="vp", bufs=8) as vppool,
            tc.tile_pool(name="big", bufs=1) as bigpool,
            tc.tile_pool(name="es", bufs=3) as espool,
            tc.tile_pool(name="ep", bufs=2) as eppool,
            tc.tile_pool(name="ps_misc", bufs=2, space="PSUM") as ps_misc,
            tc.tile_pool(name="ps_s", bufs=2, space="PSUM") as ps_s,
            tc.tile_pool(name="ps_o", bufs=1, space="PSUM") as ps_o,
        ):
            ident_b = constp.tile([64, 64], BF, name="ident_b")
            make_identity(nc, ident_b)
            ident_f = constp.tile([H + 1, H + 1], F32, name="ident_f")
            make_identity(nc, ident_f)

            # x^T chunks [128(C), 1024(T)]
            xts = []
            for c in range(NCC):
                xt = xpool.tile([128, TLOC], BF, name=f"xt{c}", tag="xt")
                nc.sync.dma_start(xt[:, :], xT[ts(c, 128), :])
                xts.append(xt)

            # weights: chunk c of W?^T at [:, c*H:(c+1)*H]
            wq_sb = wpool.tile([128, NCC * H], BF, name="wq_sb")
            wk_sb = wpool.tile([128, NCC * H], BF, name="wk_sb")
            wv_sb = wpool.tile([128, NCC * H], BF, name="wv_sb")
            for c in range(NCC):
                nc.sync.dma_start(wq_sb[:, ds(c * H, H)], wq[ts(c, 128), :])
                nc.sync.dma_start(wk_sb[:, ds(c * H, H)], wk[ts(c, 128), :])
                nc.sync.dma_start(wv_sb[:, ds(c * H, H)], wv[ts(c, 128), :])

            # projections q^T/k^T/v^T [64, 1024] (H on partitions)
            qT = projpool.tile([64, TLOC], BF, name="qT")
            kT = projpool.tile([64, TLOC], BF, name="kT")
            vT = projpool.tile([64, TLOC], BF, name="vT")
            for w_sb, dst in ((wk_sb, kT), (wv_sb, vT), (wq_sb, qT)):
                for j in range(TLOC // 512):
                    pp = ps_misc.tile([64, 512], F32, name="pp", tag="mm")
                    for c in range(NCC):
                        nc.tensor.matmul(
                            pp[:, :],
                            lhsT=w_sb[:, ds(c * H, H)],
                            rhs=xts[c][:, ts(j, 512)],
                            start=(c == 0),
                            stop=(c == NCC - 1),
                        )
                    nc.vector.tensor_copy(dst[:, ts(j, 512)], pp[:, :])

            # local contribution -> cc_in: k^T then v' (v rows + ones column)
            cc_in_k = cc_in[ds(0, KLEN)].rearrange("(h t) -> h t", h=H)
            cc_in_v = cc_in[ds(KLEN, VLEN)].rearrange("(t h) -> t h", h=H + 1)
            nc.sync.dma_start(cc_in_k[:, :], kT[:, :])
            for t in range(TLOC // 128):
                pv = ps_misc.tile([128, 64], BF, name="pv", tag="mm")
                nc.tensor.transpose(pv[:, :], vT[:, ts(t, 128)], ident_b[:, :])
                vp = vppool.tile([128, H + 1], BF, name=f"vp{t}", tag="vp")
                nc.vector.tensor_copy(vp[:, 0:H], pv[:, :])
                nc.vector.memset(vp[:, ds(H, 1)], 1.0)
                nc.sync.dma_start(cc_in_v[ts(t, 128), :], vp[:, :])

            nc.gpsimd.collective_compute(
                "AllGather",
                mybir.AluOpType.bypass,
                replica_groups=[list(range(NCORES))],
                ins=[cc_in[:]],
                outs=[cc_out[:]],
            )

            # gathered K^T [64, 8192] and V' chunks [128, 65] x 64
            kfull = bigpool.tile([64, T], BF, name="kfull")
            vg = bigpool.tile([128, NKV * (H + 1)], BF, name="vg")
            cc2 = cc_out.rearrange("(g z) -> g z", g=NCORES)
            for g in range(NCORES):
                kg = cc2[g, ds(0, KLEN)].rearrange("(h t) -> h t", h=H)
                nc.sync.dma_start(kfull[:, ts(g, TLOC)], kg[:, :])
                vgv = cc2[g, ds(KLEN, VLEN)].rearrange(
                    "(tt p h) -> tt p h", tt=TLOC // 128, p=128
                )
                for tt in range(TLOC // 128):
                    ci = g * (TLOC // 128) + tt
                    nc.sync.dma_start(vg[:, ds(ci * (H + 1), H + 1)], vgv[tt])

            # main loop over kv chunks
            obj_sb = constp.tile([128, NKV], F32, name="obj_sb")
            po = [
                ps_o.tile([H + 1, 512], F32, name=f"po{j}", tag=f"po{j}")
                for j in range(2)
            ]
            for ci in range(NKV):
                pss = ps_s.tile([128, TLOC], F32, name="pss", tag="pss")
                for j in range(2):
                    nc.tensor.matmul(
                        pss[:, ts(j, 512)],
                        lhsT=kfull[:, ts(ci, 128)],
                        rhs=qT[:, ts(j, 512)],
                        start=True,
                        stop=True,
                    )
                nc.vector.tensor_copy(obj_sb[:, ds(ci, 1)], pss[:, ds(1, 1)])
                es = espool.tile([128, TLOC], BF, name="es", tag="es")
                nc.scalar.activation(
                    es[:, :], pss[:, :], mybir.ActivationFunctionType.Exp
                )
                for j in range(2):
                    nc.tensor.matmul(
                        po[j][:, :],
                        lhsT=vg[:, ds(ci * (H + 1), H + 1)],
                        rhs=es[:, ts(j, 512)],
                        start=(ci == 0),
                        stop=(ci == NKV - 1),
                    )

            nc.sync.dma_start(obj_e[:, :], obj_sb[:, :])

            # epilogue: transpose acc back to [q, 65], divide by denominator
            for j in range(2):
                oT = eppool.tile([H + 1, 512], F32, name="oT", tag="oT")
                nc.vector.tensor_copy(oT[:, :], po[j][:, :])
                for s in range(4):
                    pt = ps_misc.tile([128, H + 1], F32, name="pt", tag="mm")
                    nc.tensor.transpose(
                        pt[:, :], oT[:, ts(s, 128)], ident_f[:, :]
                    )
                    ot = eppool.tile([128, H + 1], F32, name="ot", tag="ot")
                    nc.vector.tensor_copy(ot[:, :], pt[:, :])
                    rec = eppool.tile([128, 1], F32, name="rec", tag="rec")
                    nc.vector.reciprocal(rec[:, :], ot[:, ds(H, 1)])
                    res = eppool.tile([128, H], F32, name="res", tag="res")
                    nc.vector.tensor_scalar_mul(res[:, :], ot[:, 0:H], rec[:, :])
                    nc.sync.dma_start(
                        out_e[ds(j * 512 + s * 128, 128), :], res[:, :]
                    )

    nc.compile()
    return nc


def kernel(x, Wk, Wq, Wv):
    global LAST_EXEC_TIME_NS
    x = np.asarray(x, dtype=np.float32)
    Wk = np.asarray(Wk, dtype=np.float32)
    Wq = np.asarray(Wq, dtype=np.float32)
    Wv = np.asarray(Wv, dtype=np.float32)

    bf = ml_dtypes.bfloat16
    xTb = np.ascontiguousarray(x.T).astype(bf)            # [C, T]
    wqb = np.ascontiguousarray((Wq * SCALE).T).astype(bf)  # [C, H], scale folded
    wkb = np.ascontiguousarray(Wk.T).astype(bf)
    wvb = np.ascontiguousarray(Wv.T).astype(bf)

    if "nc" not in _CACHED:
        _CACHED["nc"] = build_nc()
    nc = _CACHED["nc"]

    in_maps = [
        {
            "xT": np.ascontiguousarray(xTb[:, g * TLOC : (g + 1) * TLOC]),
            "wqT": wqb,
            "wkT": wkb,
            "wvT": wvb,
        }
        for g in range(NCORES)
    ]

    trace = os.environ.get("KERNEL_TRACE", "1") == "1"
    if trace:
        trace = _install_profile_hook()
    r = None
    if trace:
        try:
            r = run_bass_kernel_spmd(
                nc, in_maps, core_ids=list(range(NCORES)), trace=True
            )
        except Exception as e:
            print(f"traced run failed ({e!r}); retrying untraced")
            r = None
    if r is None:
        r = run_bass_kernel_spmd(
            nc, in_maps, core_ids=list(range(NCORES)), trace=False
        )
    LAST_EXEC_TIME_NS = r.exec_time_ns
    results = r.results

    out_full = np.concatenate(
        [np.asarray(results[g]["out"], dtype=np.float32) for g in range(NCORES)],
        axis=0,
    )
    # obj buffer: [p, chunk] with affinity row 1 at kv = chunk*128 + p
    objbuf = np.asarray(results[0]["obj"], dtype=np.float32)
    obj_full = np.ascontiguousarray(objbuf.T.reshape(1, T))
    return out_full, obj_full


# revision 11
# speedup vs baseline: 1.1317x; 1.1317x over previous
"""Distributed Bass attention-head kernel for one TRN2 chip (8 NeuronCores).

Problem: x[8192,1024], Wk/Wq/Wv[64,1024] ->
  out  = softmax((x Wq^T)(x Wk^T)^T / sqrt(64)) @ (x Wv^T)   [8192, 64]
  obj  = pre-softmax affinity row 1                            [1, 8192]

Sharding: sequence-parallel. Each core owns 1024 rows of x, computes its
local q^T/k^T/v, all-gathers K^T (collective #1, launched as early as
possible) and V' (V with a ones column appended, collective #2), then runs
a flash-style loop over 64 kv chunks:
  S^T[kv=128, q=1024] = K^T_chunk^T @ q^T   (scale folded into Wq)
  P = exp(S^T)  (no max subtraction needed: |scores| <~ 8 in fp32)
  acc[65, q] += V'_chunk^T @ P              (row 64 = denominator)
The affinity row (q=1) is produced by an extra N=1 matmul per chunk so no
vector-engine read gates the scores PSUM tiles. Epilogue transposes acc
via the PE and divides by the denominator.
"""

import os

import numpy as np
import ml_dtypes

import concourse.bass as bass
import concourse.tile as tile
from concourse import bacc, mybir
from concourse.bass import ts, ds
from concourse.bass_utils import run_bass_kernel_spmd
from concourse.masks import make_identity

T, C, H = 8192, 1024, 64
NCORES = 8
TLOC = T // NCORES            # 1024 rows of x per core
SCALE = H ** -0.5
NKV = T // 128                # 64 kv chunks of 128
NCC = C // 128                # 8 contraction chunks of 128
NT = TLOC // 128              # 8 local 128-row tiles
KLEN = H * TLOC               # elems of local k^T [64, 1024]
VLEN = TLOC * (H + 1)         # elems of local v' [1024, 65]

BF = mybir.dt.bfloat16
F32 = mybir.dt.float32

LAST_EXEC_TIME_NS = None
_CACHED = {}


def _install_profile_hook():
    """Make trace=True work on the bare axon agent image.

    concourse's axon trace path reads the NTFF hook via
    ``antenv.axon_hooks``; on this image that module is absent, so
    synthesize it and register the ctypes-based hook from trn_boot.
    """
    import sys
    import types

    try:
        from antenv.axon_hooks import get_axon_ntff_profile_hook  # noqa: F401

        return True
    except ImportError:
        pass
    try:
        import antenv
        from trn_agent_boot.trn_boot import _ntff_profile_via_ctypes

        so_path = "/opt/axon/libaxon_pjrt.so"
        if not os.path.exists(so_path):
            return False
        hook = _ntff_profile_via_ctypes(so_path)
        if hook is None:
            return False
        mod = types.ModuleType("antenv.axon_hooks")
        mod._hook = hook
        mod.get_axon_ntff_profile_hook = lambda: mod._hook
        mod.set_axon_ntff_profile_hook = lambda h: setattr(mod, "_hook", h)
        sys.modules["antenv.axon_hooks"] = mod
        antenv.axon_hooks = mod
        return True
    except Exception:
        return False


def build_nc():
    nc = bacc.Bacc(None, debug=False, num_devices=NCORES)

    xT = nc.declare_dram_parameter("xT", [C, TLOC], BF, isOutput=False)
    wq = nc.declare_dram_parameter("wqT", [C, H], BF, isOutput=False)
    wk = nc.declare_dram_parameter("wkT", [C, H], BF, isOutput=False)
    wv = nc.declare_dram_parameter("wvT", [C, H], BF, isOutput=False)
    out_e = nc.declare_dram_parameter("out", [TLOC, H], F32, isOutput=True)
    obj_e = nc.declare_dram_parameter("obj", [128, NKV], F32, isOutput=True)

    cck_in = nc.dram_tensor("cck_in", [KLEN], BF)
    cck_out = nc.dram_tensor("cck_out", [NCORES * KLEN], BF, addr_space="Shared")
    ccv_in = nc.dram_tensor("ccv_in", [VLEN], BF)
    ccv_out = nc.dram_tensor("ccv_out", [NCORES * VLEN], BF, addr_space="Shared")

    with tile.TileContext(nc) as tc:
        with (
            tc.tile_pool(name="const", bufs=1) as constp,
            tc.tile_pool(name="xts", bufs=NCC) as xpool,
            tc.tile_pool(name="wts", bufs=1) as wpool,
            tc.tile_pool(name="proj", bufs=1) as projpool,
            tc.tile_pool(name="big", bufs=1) as bigpool,
            tc.tile_pool(name="es", bufs=12) as espool,
            tc.tile_pool(name="ep", bufs=2) as eppool,
            tc.tile_pool(name="ps_misc", bufs=1, space="PSUM") as ps_misc,
            tc.tile_pool(name="ps_s", bufs=2, space="PSUM") as ps_s,
            tc.tile_pool(name="ps_acc", bufs=1, space="PSUM") as ps_acc,
        ):
            # weights first (k projection is the critical path to collective #1)
            wk_sb = wpool.tile([128, NCC * H], BF, name="wk_sb")
            wv_sb = wpool.tile([128, NCC * H], BF, name="wv_sb")
            wq_sb = wpool.tile([128, NCC * H], BF, name="wq_sb")
            for w_sb, w_ext in ((wk_sb, wk), (wv_sb, wv), (wq_sb, wq)):
                nc.sync.dma_start(
                    w_sb.rearrange("p (c h) -> p c h", c=NCC),
                    w_ext.rearrange("(c p) h -> p c h", p=128),
                )

            # x^T chunks [128(C), 1024(T)]
            xts = []
            for c in range(NCC):
                xt = xpool.tile([128, TLOC], BF, name=f"xt{c}", tag="xt")
                nc.scalar.dma_start(xt[:, :], xT[ts(c, 128), :])
                xts.append(xt)

            ident_b = constp.tile([64, 64], BF, name="ident_b")
            make_identity(nc, ident_b)
            ident_f = constp.tile([H + 1, H + 1], F32, name="ident_f")
            make_identity(nc, ident_f)

            def project(w_sb, dst):
                for j in range(TLOC // 512):
                    pp = ps_misc.tile([64, 512], F32, name="pp", tag="mm")
                    for c in range(NCC):
                        nc.tensor.matmul(
                            pp[:, :],
                            lhsT=w_sb[:, ds(c * H, H)],
                            rhs=xts[c][:, ts(j, 512)],
                            start=(c == 0),
                            stop=(c == NCC - 1),
                        )
                    nc.vector.tensor_copy(dst[:, ts(j, 512)], pp[:, :])

            # k^T first -> stage -> collective #1
            kT = projpool.tile([64, TLOC], BF, name="kT")
            project(wk_sb, kT)
            cck_in_v = cck_in.rearrange("(h t) -> h t", h=H)
            nc.sync.dma_start(cck_in_v[:, :], kT[:, :])
            nc.gpsimd.collective_compute(
                "AllGather",
                mybir.AluOpType.bypass,
                replica_groups=[list(range(NCORES))],
                ins=[cck_in[:]],
                outs=[cck_out[:]],
            )

            # v^T -> transpose to v tiles + ones column -> collective #2
            vT = projpool.tile([64, TLOC], BF, name="vT")
            project(wv_sb, vT)
            vpall = constp.tile([128, NT * (H + 1)], BF, name="vpall")
            for t in range(NT):
                pv = ps_misc.tile([128, 64], BF, name="pv", tag="mm")
                nc.tensor.transpose(pv[:, :], vT[:, ts(t, 128)], ident_b[:, :])
                nc.vector.tensor_copy(vpall[:, ds(t * (H + 1), H)], pv[:, :])
                nc.vector.memset(vpall[:, ds(t * (H + 1) + H, 1)], 1.0)
            ccv_in_v = ccv_in.rearrange("(tt p h) -> p tt h", p=128, h=H + 1)
            nc.sync.dma_start(
                ccv_in_v, vpall.rearrange("p (tt h) -> p tt h", tt=NT)
            )
            nc.gpsimd.collective_compute(
                "AllGather",
                mybir.AluOpType.bypass,
                replica_groups=[list(range(NCORES))],
                ins=[ccv_in[:]],
                outs=[ccv_out[:]],
            )

            # q^T (only needed once K^T arrives)
            qT = projpool.tile([64, TLOC], BF, name="qT")
            project(wq_sb, qT)

            # gathered K^T [64, 8192]: cck_out is [g, h, t] -> [h, (g t)]
            kfull = bigpool.tile([64, T], BF, name="kfull")
            nc.scalar.dma_start(
                kfull.rearrange("h (g t) -> h g t", g=NCORES),
                cck_out.rearrange("(g h t) -> h g t", g=NCORES, h=H),
            )
            # gathered V' [8192, 65] as 64 chunks [128, 65]
            vg = bigpool.tile([128, NKV * (H + 1)], BF, name="vg")
            nc.scalar.dma_start(
                vg.rearrange("p (ci h) -> p ci h", h=H + 1),
                ccv_out.rearrange("(ci p h) -> p ci h", p=128, h=H + 1),
            )

            # main loop over kv chunks
            po = [
                ps_acc.tile([H + 1, 512], F32, name=f"po{j}", tag=f"po{j}")
                for j in range(2)
            ]
            obj_ps = ps_acc.tile([128, NKV], F32, name="obj_ps", tag="obj")
            for ci in range(NKV):
                pss = ps_s.tile([128, TLOC], F32, name="pss", tag="pss")
                for j in range(2):
                    nc.tensor.matmul(
                        pss[:, ts(j, 512)],
                        lhsT=kfull[:, ts(ci, 128)],
                        rhs=qT[:, ts(j, 512)],
                        start=True,
                        stop=True,
                    )
                # affinity row q=1, pre-softmax, via a cheap N=1 matmul
                nc.tensor.matmul(
                    obj_ps[:, ds(ci, 1)],
                    lhsT=kfull[:, ts(ci, 128)],
                    rhs=qT[:, ds(1, 1)],
                    start=(ci == 0),
                    stop=(ci == NKV - 1),
                )
                es = espool.tile([128, TLOC], BF, name="es", tag="es")
                nc.scalar.activation(
                    es[:, :], pss[:, :], mybir.ActivationFunctionType.Exp
                )
                for j in range(2):
                    nc.tensor.matmul(
                        po[j][:, :],
                        lhsT=vg[:, ds(ci * (H + 1), H + 1)],
                        rhs=es[:, ts(j, 512)],
                        start=(ci == 0),
                        stop=(ci == NKV - 1),
                    )

            obj_sb = constp.tile([128, NKV], F32, name="obj_sb")
            nc.vector.tensor_copy(obj_sb[:, :], obj_ps[:, :])
            nc.sync.dma_start(obj_e[:, :], obj_sb[:, :])

            # epilogue: transpose acc back to [q, 65], divide by denominator
            for j in range(2):
                oT = eppool.tile([H + 1, 512], F32, name="oT", tag="oT")
                nc.vector.tensor_copy(oT[:, :], po[j][:, :])
                for s in range(4):
                    pt = ps_misc.tile([128, H + 1], F32, name="pt", tag="mm")
                    nc.tensor.transpose(pt[:, :], oT[:, ts(s, 128)], ident_f[:, :])
                    ot = eppool.tile([128, H + 1], F32, name="ot", tag="ot")
                    nc.vector.tensor_copy(ot[:, :], pt[:, :])
                    rec = eppool.tile([128, 1], F32, name="rec", tag="rec")
                    nc.vector.reciprocal(rec[:, :], ot[:, ds(H, 1)])
                    res = eppool.tile([128, H], F32, name="res", tag="res")
                    nc.vector.tensor_scalar_mul(res[:, :], ot[:, 0:H], rec[:, :])
                    nc.sync.dma_start(
                        out_e[ds(j * 512 + s * 128, 128), :], res[:, :]
                    )

    nc.compile()
    return nc


def kernel(x, Wk, Wq, Wv):
    global LAST_EXEC_TIME_NS
    x = np.asarray(x, dtype=np.float32)
    Wk = np.asarray(Wk, dtype=np.float32)
    Wq = np.asarray(Wq, dtype=np.float32)
    Wv = np.asarray(Wv, dtype=np.float32)

    bf = ml_dtypes.bfloat16
    xTb = np.ascontiguousarray(x.T).astype(bf)             # [C, T]
    wqb = np.ascontiguousarray((Wq * SCALE).T).astype(bf)  # [C, H], scale folded
    wkb = np.ascontiguousarray(Wk.T).astype(bf)
    wvb = np.ascontiguousarray(Wv.T).astype(bf)

    if "nc" not in _CACHED:
        _CACHED["nc"] = build_nc()
    nc = _CACHED["nc"]

    in_maps = [
        {
            "xT": np.ascontiguousarray(xTb[:, g * TLOC : (g + 1) * TLOC]),
            "wqT": wqb,
            "wkT": wkb,
            "wvT": wvb,
        }
        for g in range(NCORES)
    ]

    trace = os.environ.get("KERNEL_TRACE", "1") == "1"
    if trace:
        trace = _install_profile_hook()
    r = None
    if trace:
        try:
            r = run_bass_kernel_spmd(
                nc, in_maps, core_ids=list(range(NCORES)), trace=True
            )
        except Exception as e:
            print(f"traced run failed ({e!r}); retrying untraced")
            r = None
    if r is None:
        r = run_bass_kernel_spmd(
            nc, in_maps, core_ids=list(range(NCORES)), trace=False
        )
    LAST_EXEC_TIME_NS = r.exec_time_ns
    results = r.results

    out_full = np.concatenate(
        [np.asarray(results[g]["out"], dtype=np.float32) for g in range(NCORES)],
        axis=0,
    )
    # obj buffer: [p, chunk] with affinity row 1 at kv = chunk*128 + p
    objbuf = np.asarray(results[0]["obj"], dtype=np.float32)
    obj_full = np.ascontiguousarray(objbuf.T.reshape(1, T))
    return out_full, obj_full
